# revision 4
# baseline (speedup 1.0000x reference)
"""Trainium2 Bass kernel for nn_EquivariantUpdate (GNN message-passing
equivariant coordinate update), distributed over 8 NeuronCores.

Math (per reference):
    inp  = [h[row], h[col], edge_attr]            # [E, 257]
    x    = silu(inp @ W1 + b1); x = silu(x @ W2 + b2)
    phi  = x @ W3                                  # [E, 1]
    trans= coord_diff * phi * edge_mask            # [E, 3]
    agg  = segment_sum(trans, row, N) / 100
    out  = (coord + agg) * node_mask

Sharding: edges are sorted by destination node (row) on the host and nodes are
split into 128-node chunks; each core owns 49 consecutive chunks and all edges
whose row lands in them, so per-core partial sums are complete — no collective
is needed. h / weights are replicated.

Per-core device pipeline:
  phase 0: A = h_slice @ W1[:128]  (own node slice),  B = h @ W1[128:256]
           (full table), both written to DRAM in bf16 (layer-1 factorization:
           silu1 input X1[e] = A[row[e]] + B[col[e]] + edge_attr[e]*W1c + b1).
  phase 1: per node chunk: indirect-DMA gather A[row]/B[col], assemble X1 on
           DVE, PE-transpose to [hidden, edge] layout, SiLU (+b1) on ACT,
           X2T = W2.T @ XT, SiLU (+b2), phi = X2T.T @ W3, trans = cd * phi,
           segment-sum via selection-matrix matmul S.T @ trans accumulated in
           PSUM, then (agg/100 + coord) * node_mask.

edge_mask is folded into the row-local index (masked/padded edges get -1,
which never matches the iota compare, so they contribute exactly 0).
"""

import json

import ml_dtypes
import numpy as np

import concourse.bass as bass
import concourse.bass2jax as bass2jax
import concourse.mybir as mybir
import concourse.tile as tile
from concourse.bass_utils import run_bass_kernel_spmd

# ---------------------------------------------------------------------------
# BIR patch: this walrus build's codegen accepts only ONE sync-wait command
# per instruction encoding slot; Tile's kernel-tail Drain (and occasionally
# other instructions) carry more. Split overflow waits onto inserted Drains.
# ---------------------------------------------------------------------------
_MAX_WAITS = 1
_orig_compile_bir = bass2jax.compile_bir_kernel


def _split_waits(bir: dict) -> int:
    n = 0
    for fn in bir.get("functions", []):
        for blk in fn.get("blocks", []):
            out = []
            for ins in blk.get("instructions", []):
                si = ins.get("sync_info") or {}
                waits = si.get("on_wait") or []
                if len(waits) > _MAX_WAITS:
                    extra, keep = waits[:-_MAX_WAITS], waits[-_MAX_WAITS:]
                    for ci in range(0, len(extra), _MAX_WAITS):
                        out.append({
                            "debug": ins.get("debug", 0),
                            "engine": ins["engine"],
                            "ins": [],
                            "is_reset_sema": False,
                            "name": f"{ins['name']}-wsplit{ci}",
                            "opcode": "Drain",
                            "outs": [],
                            "sync_info": {
                                "on_update": [],
                                "on_wait": extra[ci : ci + _MAX_WAITS],
                            },
                        })
                    si["on_wait"] = keep
                    n += 1
                out.append(ins)
            blk["instructions"] = out
    return n


def _patched_compile_bir(bir_json: bytes, tmpdir: str, neff_name="file.neff") -> str:
    bir = json.loads(bir_json)
    if _split_waits(bir):
        bir_json = json.dumps(bir).encode()
    return _orig_compile_bir(bir_json, tmpdir, neff_name)


bass2jax.compile_bir_kernel = _patched_compile_bir

# ---------------------------------------------------------------------------
# Problem constants (hardcoded per spec)
# ---------------------------------------------------------------------------
N_NODES = 50000
N_EDGES = 800000
H = 128
NORM = 100.0
NCORES = 8
CH = 128                      # nodes per chunk
CPC = 49                      # chunks per core
NCH = NCORES * CPC            # 392 chunks (391 real + 1 dummy)
NSL = CPC * CH                # 6272 nodes per core slice
NPADF = 391 * CH              # 50048: padded node count for full tables
NPADA = NCH * CH              # 50176: padding for per-core slicing

BF = mybir.dt.bfloat16
F32 = mybir.dt.float32
I32 = mybir.dt.int32
NP_BF = ml_dtypes.bfloat16

DT_G = BF                     # dtype of A/B gather tables
NP_G = NP_BF

# tracing knobs (used by test.py; harness leaves these off)
TRACE = False
TRACE_DIR = None
TRACE_CORES = None
LAST_RESULT = None


def _build_program(bmax: int):
    EC = CH * bmax            # padded edges per chunk
    nc = bass.Bass()

    h_full = nc.declare_dram_parameter("h_full", [NPADF, H], BF, isOutput=False)
    h_slice = nc.declare_dram_parameter("h_slice", [NSL, H], BF, isOutput=False)
    w1ab = nc.declare_dram_parameter("w1ab", [H, 2 * H], BF, isOutput=False)
    w2 = nc.declare_dram_parameter("w2", [H, H], BF, isOutput=False)
    w3 = nc.declare_dram_parameter("w3", [H, 1], BF, isOutput=False)
    w1c_bc = nc.declare_dram_parameter("w1c_bc", [128, H], BF, isOutput=False)
    iota_bc = nc.declare_dram_parameter("iota_bc", [128, 128], BF, isOutput=False)
    ident = nc.declare_dram_parameter("ident", [128, 128], BF, isOutput=False)
    b1 = nc.declare_dram_parameter("b1", [128, 1], F32, isOutput=False)
    b2 = nc.declare_dram_parameter("b2", [128, 1], F32, isOutput=False)
    idxp = nc.declare_dram_parameter("idxp", [CPC, 128, 2 * bmax], I32, isOutput=False)
    rowlocp = nc.declare_dram_parameter("rowlocp", [CPC, 128, bmax], BF, isOutput=False)
    attrp = nc.declare_dram_parameter("attrp", [CPC, 128, bmax], BF, isOutput=False)
    cdp = nc.declare_dram_parameter("cdp", [CPC, 128, 3 * bmax], BF, isOutput=False)
    coordl = nc.declare_dram_parameter("coordl", [128, 3 * CPC], F32, isOutput=False)
    nmaskl = nc.declare_dram_parameter("nmaskl", [128, 3 * CPC], F32, isOutput=False)
    out = nc.declare_dram_parameter("out", [128, 3 * CPC], F32, isOutput=True)

    with tile.TileContext(nc) as tc:
        with (
            tc.tile_pool(name="const", bufs=1) as cpool,
            tc.tile_pool(name="dram", bufs=1, space="DRAM") as dpool,
        ):
            w1ab_sb = cpool.tile([H, 2 * H], BF)
            nc.sync.dma_start(out=w1ab_sb[:], in_=w1ab[:])
            w2_sb = cpool.tile([H, H], BF)
            nc.sync.dma_start(out=w2_sb[:], in_=w2[:])
            w3_sb = cpool.tile([H, 1], BF)
            nc.sync.dma_start(out=w3_sb[:], in_=w3[:])
            w1c_sb = cpool.tile([128, H], BF)
            nc.sync.dma_start(out=w1c_sb[:], in_=w1c_bc[:])
            iota_sb = cpool.tile([128, 128], BF)
            nc.sync.dma_start(out=iota_sb[:], in_=iota_bc[:])
            ident_sb = cpool.tile([128, 128], BF)
            nc.sync.dma_start(out=ident_sb[:], in_=ident[:])
            b1_sb = cpool.tile([128, 1], F32)
            nc.sync.dma_start(out=b1_sb[:], in_=b1[:])
            b2_sb = cpool.tile([128, 1], F32)
            nc.sync.dma_start(out=b2_sb[:], in_=b2[:])
            coord_sb = cpool.tile([128, 3 * CPC], F32)
            nc.sync.dma_start(out=coord_sb[:], in_=coordl[:])
            nmask_sb = cpool.tile([128, 3 * CPC], F32)
            nc.sync.dma_start(out=nmask_sb[:], in_=nmaskl[:])
            agg_all = cpool.tile([128, 3 * CPC], F32)

            a_dram = dpool.tile([NSL, H], DT_G)
            b_dram = dpool.tile([NPADF, H], DT_G)

            # ---- phase 0: A (own slice) and B (full) tables ----
            with (
                tc.tile_pool(name="p0", bufs=3) as p0,
                tc.tile_pool(name="p0ps", bufs=2, space="PSUM") as p0ps,
            ):
                for k in range(CPC):
                    hT = p0.tile([128, 128], BF, tag="hT")
                    nc.sync.dma_start_transpose(
                        out=hT[:], in_=h_slice[k * 128 : (k + 1) * 128, :]
                    )
                    ps = p0ps.tile([128, 128], F32, tag="ps")
                    nc.tensor.matmul(
                        out=ps[:], lhsT=hT[:], rhs=w1ab_sb[:, 0:H],
                        start=True, stop=True,
                    )
                    ab = p0.tile([128, 128], DT_G, tag="ab")
                    nc.vector.tensor_copy(out=ab[:], in_=ps[:])
                    nc.sync.dma_start(
                        out=a_dram[k * 128 : (k + 1) * 128, :], in_=ab[:]
                    )
                for k in range(391):
                    hT = p0.tile([128, 128], BF, tag="hT")
                    nc.sync.dma_start_transpose(
                        out=hT[:], in_=h_full[k * 128 : (k + 1) * 128, :]
                    )
                    ps = p0ps.tile([128, 128], F32, tag="ps")
                    nc.tensor.matmul(
                        out=ps[:], lhsT=hT[:], rhs=w1ab_sb[:, H : 2 * H],
                        start=True, stop=True,
                    )
                    ab = p0.tile([128, 128], DT_G, tag="ab")
                    nc.vector.tensor_copy(out=ab[:], in_=ps[:])
                    nc.sync.dma_start(
                        out=b_dram[k * 128 : (k + 1) * 128, :], in_=ab[:]
                    )

            # ---- phase 1: per-chunk edge pipeline ----
            with (
                tc.tile_pool(name="p1", bufs=2) as pool,
                tc.tile_pool(name="pps", bufs=1, space="PSUM") as pps,
                tc.tile_pool(name="pps2", bufs=1, space="PSUM") as pps2,
            ):
                for k in range(CPC):
                    idx_t = pool.tile([128, 2 * bmax], I32, tag="idx")
                    nc.sync.dma_start(out=idx_t[:], in_=idxp[k])
                    rl_t = pool.tile([128, bmax], BF, tag="rl")
                    nc.sync.dma_start(out=rl_t[:], in_=rowlocp[k])
                    at_t = pool.tile([128, bmax], BF, tag="at")
                    nc.sync.dma_start(out=at_t[:], in_=attrp[k])
                    cd_t = pool.tile([128, 3 * bmax], BF, tag="cd")
                    nc.sync.dma_start(out=cd_t[:], in_=cdp[k])

                    aga = pool.tile([128, EC], DT_G, tag="aga")
                    bga = pool.tile([128, EC], DT_G, tag="bga")
                    for b in range(bmax):
                        nc.gpsimd.indirect_dma_start(
                            out=aga[:, b * 128 : (b + 1) * 128],
                            out_offset=None,
                            in_=a_dram[:],
                            in_offset=bass.IndirectOffsetOnAxis(
                                ap=idx_t[:, 2 * b : 2 * b + 1], axis=0
                            ),
                        )
                        nc.gpsimd.indirect_dma_start(
                            out=bga[:, b * 128 : (b + 1) * 128],
                            out_offset=None,
                            in_=b_dram[:],
                            in_offset=bass.IndirectOffsetOnAxis(
                                ap=idx_t[:, 2 * b + 1 : 2 * b + 2], axis=0
                            ),
                        )

                    # X1 = A[row] + B[col] + attr * W1c   (bf16, [e, j] layout)
                    x1 = pool.tile([128, EC], BF, tag="x1")
                    nc.vector.tensor_add(out=x1[:], in0=aga[:], in1=bga[:])
                    t2 = pool.tile([128, EC], BF, tag="t2")
                    nc.vector.tensor_tensor(
                        out=t2[:].rearrange("p (b j) -> p b j", b=bmax),
                        in0=at_t[:].to_broadcast([128, bmax, H]),
                        in1=w1c_sb[:].rearrange("p (b j) -> p b j", b=1)
                        .to_broadcast([128, bmax, H]),
                        op=mybir.AluOpType.mult,
                    )
                    nc.vector.tensor_add(out=x1[:], in0=x1[:], in1=t2[:])

                    # transpose each block -> X1T [j, e] (bf16 PSUM)
                    x1t = pps.tile([128, EC], BF, tag="xt_ps")
                    for b in range(bmax):
                        nc.tensor.transpose(
                            out=x1t[:, b * 128 : (b + 1) * 128],
                            in_=x1[:, b * 128 : (b + 1) * 128],
                            identity=ident_sb[:],
                        )
                    xt = pool.tile([128, EC], BF, tag="xt_sb")
                    nc.scalar.activation(
                        out=xt[:], in_=x1t[:],
                        func=mybir.ActivationFunctionType.Silu,
                        bias=b1_sb[:, :1],
                    )

                    # layer 2: X2T = W2.T @ XT  (+ silu + b2)
                    x2t = pps.tile([128, EC], F32, tag="xt_ps")
                    for s in range(0, EC, 512):
                        e = min(s + 512, EC)
                        nc.tensor.matmul(
                            out=x2t[:, s:e], lhsT=w2_sb[:], rhs=xt[:, s:e],
                            start=True, stop=True,
                        )
                    x2ts = pool.tile([128, EC], BF, tag="x2t_sb")
                    nc.scalar.activation(
                        out=x2ts[:], in_=x2t[:],
                        func=mybir.ActivationFunctionType.Silu,
                        bias=b2_sb[:, :1],
                    )

                    # phi[e] = X2T.T @ W3  (per block, [e, 1] PSUM columns)
                    phi = pps2.tile([128, bmax], F32, tag="phi")
                    for b in range(bmax):
                        nc.tensor.matmul(
                            out=phi[:, b : b + 1],
                            lhsT=x2ts[:, b * 128 : (b + 1) * 128],
                            rhs=w3_sb[:],
                            start=True, stop=True,
                        )

                    # S[e, n] = (rowloc[e] == n);  trans = cd * phi
                    s_t = pool.tile([128, EC], BF, tag="s")
                    nc.vector.tensor_tensor(
                        out=s_t[:].rearrange("p (b j) -> p b j", b=bmax),
                        in0=rl_t[:].to_broadcast([128, bmax, 128]),
                        in1=iota_sb[:].rearrange("p (b j) -> p b j", b=1)
                        .to_broadcast([128, bmax, 128]),
                        op=mybir.AluOpType.is_equal,
                    )
                    trans = pool.tile([128, 3 * bmax], BF, tag="trans")
                    nc.vector.tensor_tensor(
                        out=trans[:].rearrange("p (b c) -> p b c", b=bmax),
                        in0=cd_t[:].rearrange("p (b c) -> p b c", b=bmax),
                        in1=phi[:].to_broadcast([128, bmax, 3]),
                        op=mybir.AluOpType.mult,
                    )

                    # agg[n, :] = sum_b S_b.T @ trans_b   (PSUM accumulation)
                    agg = pps2.tile([128, 3], F32, tag="agg")
                    for b in range(bmax):
                        nc.tensor.matmul(
                            out=agg[:],
                            lhsT=s_t[:, b * 128 : (b + 1) * 128],
                            rhs=trans[:, 3 * b : 3 * b + 3],
                            start=(b == 0), stop=(b == bmax - 1),
                        )
                    nc.vector.tensor_scalar_mul(
                        out=agg_all[:, 3 * k : 3 * k + 3], in0=agg[:],
                        scalar1=1.0 / NORM,
                    )

                # out = (agg/norm + coord) * node_mask
                out_sb = pool.tile([128, 3 * CPC], F32, tag="outsb")
                nc.vector.tensor_add(out=out_sb[:], in0=agg_all[:], in1=coord_sb[:])
                nc.vector.tensor_mul(out=out_sb[:], in0=out_sb[:], in1=nmask_sb[:])
                nc.sync.dma_start(out=out[:], in_=out_sb[:])

    return nc


def kernel(**inputs: np.ndarray) -> np.ndarray:
    h = np.asarray(inputs["h"], dtype=np.float32)
    coord = np.asarray(inputs["coord"], dtype=np.float32)
    edge_index = np.asarray(inputs["edge_index"]).astype(np.int64)
    coord_diff = np.asarray(inputs["coord_diff"], dtype=np.float32)
    edge_attr = np.asarray(inputs["edge_attr"], dtype=np.float32)
    node_mask = np.asarray(inputs["node_mask"], dtype=np.float32)
    edge_mask = np.asarray(inputs["edge_mask"], dtype=np.float32)
    W1 = np.asarray(inputs["W1"], dtype=np.float32)
    b1 = np.asarray(inputs["b1"], dtype=np.float32)
    W2 = np.asarray(inputs["W2"], dtype=np.float32)
    b2 = np.asarray(inputs["b2"], dtype=np.float32)
    W3 = np.asarray(inputs["W3"], dtype=np.float32)

    E = edge_index.shape[1]
    row, col = edge_index[0], edge_index[1]

    # sort edges by destination node
    perm = np.argsort(row, kind="stable")
    rs = row[perm]
    cs = col[perm]
    cds = coord_diff[perm]
    ats = edge_attr[perm, 0]
    ems = edge_mask[perm, 0]

    chunk_of = rs // CH
    starts = np.searchsorted(rs, np.arange(NCH + 1) * CH)
    cnts = np.diff(starts)
    bmax = max(1, int(-(-cnts.max() // 128)))
    EC = CH * bmax

    # padded per-chunk edge arrays (slot = chunk * EC + position-in-chunk)
    slot = chunk_of * EC + (np.arange(E) - starts[chunk_of])
    rowloc_g = np.full(NCH * EC, -1.0, np.float32)
    rowloc_g[slot] = np.where(ems != 0, (rs - chunk_of * CH).astype(np.float32), -1.0)
    idxr_g = np.full(NCH * EC, -1, np.int64)
    idxr_g[slot] = rs
    idxc_g = np.zeros(NCH * EC, np.int64)
    idxc_g[slot] = cs
    attr_g = np.zeros(NCH * EC, np.float32)
    attr_g[slot] = ats
    cd_g = np.zeros((NCH * EC, 3), np.float32)
    cd_g[slot] = cds

    # device layouts: [chunk, partition(e%128), block]
    rowloc_d = rowloc_g.reshape(NCH, bmax, 128).transpose(0, 2, 1).astype(NP_BF)
    attr_d = attr_g.reshape(NCH, bmax, 128).transpose(0, 2, 1).astype(NP_BF)
    cd_d = (
        cd_g.reshape(NCH, bmax, 128, 3).transpose(0, 2, 1, 3)
        .reshape(NCH, 128, 3 * bmax).astype(NP_BF)
    )
    idxr_d = idxr_g.reshape(NCH, bmax, 128).transpose(0, 2, 1)
    idxc_d = idxc_g.reshape(NCH, bmax, 128).transpose(0, 2, 1)

    h_pad = np.zeros((NPADA, H), np.float32)
    h_pad[:N_NODES] = h
    h_bf = h_pad.astype(NP_BF)
    coord_pad = np.zeros((NPADA, 3), np.float32)
    coord_pad[:N_NODES] = coord
    nmask_pad = np.zeros((NPADA, 1), np.float32)
    nmask_pad[:N_NODES] = node_mask

    # W1a = W1[:128], W1b = W1[128:256] (both [in, out]); concat along out cols
    w1ab_np = np.concatenate([W1[:H], W1[H : 2 * H]], axis=1).astype(NP_BF)
    w2_np = W2.astype(NP_BF)
    w3_np = W3.reshape(H, 1).astype(NP_BF)
    w1c_np = np.tile(W1[2 * H].reshape(1, H), (128, 1)).astype(NP_BF)
    iota_np = np.tile(np.arange(128, dtype=np.float32), (128, 1)).astype(NP_BF)
    ident_np = np.eye(128, dtype=np.float32).astype(NP_BF)
    b1_np = b1.reshape(H, 1).astype(np.float32)
    b2_np = b2.reshape(H, 1).astype(np.float32)

    nc = _build_program(bmax)

    in_maps = []
    for i in range(NCORES):
        c0 = i * CPC
        n0 = c0 * CH
        sl = slice(c0, c0 + CPC)
        idxr_i = idxr_d[sl] - n0
        idxr_i[idxr_d[sl] < 0] = 0
        idx_i = np.stack([idxr_i, idxc_d[sl]], axis=3).reshape(
            CPC, 128, 2 * bmax
        ).astype(np.int32)
        coordl = (
            coord_pad[n0 : n0 + NSL].reshape(CPC, 128, 3).transpose(1, 0, 2)
            .reshape(128, 3 * CPC).copy()
        )
        nmaskl = (
            np.repeat(nmask_pad[n0 : n0 + NSL], 3, axis=1)
            .reshape(CPC, 128, 3).transpose(1, 0, 2).reshape(128, 3 * CPC).copy()
        )
        in_maps.append({
            "h_full": np.ascontiguousarray(h_bf[:NPADF]),
            "h_slice": np.ascontiguousarray(h_bf[n0 : n0 + NSL]),
            "w1ab": w1ab_np, "w2": w2_np, "w3": w3_np,
            "w1c_bc": w1c_np, "iota_bc": iota_np, "ident": ident_np,
            "b1": b1_np, "b2": b2_np,
            "idxp": np.ascontiguousarray(idx_i),
            "rowlocp": np.ascontiguousarray(rowloc_d[sl]),
            "attrp": np.ascontiguousarray(attr_d[sl]),
            "cdp": np.ascontiguousarray(cd_d[sl]),
            "coordl": coordl, "nmaskl": nmaskl,
        })

    kwargs = {}
    if TRACE:
        kwargs = dict(trace=True, tmpdir=TRACE_DIR, trace_cores=TRACE_CORES)
    res = run_bass_kernel_spmd(nc, in_maps, core_ids=list(range(NCORES)), **kwargs)
    global LAST_RESULT
    LAST_RESULT = res

    out_full = np.zeros((NPADA, 3), np.float32)
    for i in range(NCORES):
        o = res.results[i]["out"]  # [128, 3*CPC]
        o = o.reshape(128, CPC, 3).transpose(1, 0, 2).reshape(NSL, 3)
        out_full[i * NSL : (i + 1) * NSL] = o
    return out_full[:N_NODES].astype(np.float32)
